# revision 17
# baseline (speedup 1.0000x reference)
"""GAT (2x GATConv + global_mean_pool + MLP) on 8 Trainium2 NeuronCores.

v2 design (vs baseline):
  - All matmul/elementwise data in bf16 (PE 1 cyc/row vs fp32's 4).
  - Per-tile dma_gather (InstDMAGatherAnt, one launch per table per tile)
    replaces per-block indirect DMAs (~1us SWDGE overhead each).
  - L1 aggregation in transposed orientation: A_hT[c,s] accumulates via
    lhsT = gathered x block, rhs = per-head scaled one-hots; no transpose
    round before the W1 matmuls.
  - dst-side attention logits gathered from small local padded tables.
  - elu via Exp + 2 fused DVE ops; leaky-relu via one scalar_tensor_tensor.
  - AllGather of the [1280,576] bf16 h2p|a2src table split in halves to
    overlap with L1 tile compute.
Node->slot assignment is load-balanced on the host (LPT over 80 dst
tiles) so every (core,tile) has <= bt*128 edges with bt minimal.
"""
import os
import sys
import numpy as np

for _p in ("/opt/trn_rl_repo",):
    if os.path.isdir(_p) and _p not in sys.path:
        sys.path.insert(0, _p)

import ml_dtypes

N = 10000
B = 16
NCORES = 8
P = 128
TPC = 10                    # dst tiles per core
SPC = TPC * P               # slots per core (1280)
NSLOT = NCORES * SPC        # 10240
HALF = SPC // 2             # 640 rows per AG half
NEG = 0.2
XW = 256                    # xs table row (x:128 | asrc:8 | pad) bf16 -> 512B
DW = 128                    # dst-table padded row bf16 -> 256B
HW = 640                    # h2p table row (h2p:512 | a2src:8 | pad) -> 1280B

_PROGRAM_CACHE = {}
LAST_PROFILE = {}

bf = ml_dtypes.bfloat16


def _preprocess(edge_index, batch):
    src = np.concatenate([np.asarray(edge_index[0]), np.arange(N)]).astype(np.int64)
    dst = np.concatenate([np.asarray(edge_index[1]), np.arange(N)]).astype(np.int64)
    batch = np.asarray(batch).astype(np.int64)

    deg = np.bincount(dst, minlength=N)

    # LPT: nodes (by in-degree desc) -> 80 bins of <=128 nodes, then bins ->
    # cores (10 bins each) balancing core edge totals.
    import heapq
    order = np.argsort(-deg, kind='stable')
    nbins = NCORES * TPC
    heap = [(0, b) for b in range(nbins)]
    heapq.heapify(heap)
    bin_nodes = [[] for _ in range(nbins)]
    bin_load = np.zeros(nbins, np.int64)
    spill = []
    for n in order:
        load, b = heapq.heappop(heap)
        bin_nodes[b].append(n)
        bin_load[b] += deg[n]
        if len(bin_nodes[b]) < P:
            heapq.heappush(heap, (bin_load[b], b))
        else:
            spill.append(b)
    # bins -> cores: LPT on bin loads into 8 groups of exactly TPC bins
    border = np.argsort(-bin_load, kind='stable')
    cheap = [(0, TPC, c) for c in range(NCORES)]
    core_bins = [[] for _ in range(NCORES)]
    cload = np.zeros(NCORES, np.int64)
    ccap = [TPC] * NCORES
    import heapq as hq
    ch = [(0, c) for c in range(NCORES)]
    hq.heapify(ch)
    for b in border:
        while True:
            load, c = hq.heappop(ch)
            if ccap[c] > 0:
                break
        core_bins[c].append(b)
        cload[c] += bin_load[b]
        ccap[c] -= 1
        if ccap[c] > 0:
            hq.heappush(ch, (cload[c], c))

    # slot assignment: core c, tile t, slot index within tile by bin order
    slot_of = np.full(N, -1, np.int64)      # global slot 0..NSLOT-1
    for c in range(NCORES):
        for t, b in enumerate(core_bins[c]):
            base = c * SPC + t * P
            for i, n in enumerate(bin_nodes[b]):
                slot_of[n] = base + i
    assert (slot_of >= 0).all()

    owner = slot_of // SPC
    local = slot_of % SPC                     # 0..1279 within owner core
    # std layout row (xs_tab, phase-A order)
    l2std = slot_of
    # h2p_full layout row (single AllGather): owner*1280 + local
    l2ag = slot_of

    # per-edge, grouped by dst (core,tile)
    dslot = slot_of[dst]
    dcore = dslot // SPC
    dtile = (dslot % SPC) // P
    dseg = (dslot % P).astype(np.float32)

    counts = np.zeros((NCORES, TPC), np.int64)
    np.add.at(counts, (dcore, dtile), 1)
    bt = max(int(np.ceil(counts.max() / P)), 1)
    NI = bt * P

    eorder = np.lexsort((src, dtile, dcore))
    src_s, dseg_s = src[eorder], dseg[eorder]
    dcore_s, dtile_s = dcore[eorder], dtile[eorder]
    dloc_s = (slot_of[dst] % SPC)[eorder]

    grp = dcore_s * TPC + dtile_s
    gstart = np.searchsorted(grp, np.arange(NCORES * TPC), 'left')
    rank = np.arange(len(grp)) - gstart[grp]

    # flat edge position k = rank; (p, b) = (k % 128, k // 128)
    srcl1 = np.zeros((NCORES, TPC, NI), np.int16)
    srcl2 = np.zeros((NCORES, TPC, NI), np.int16)
    dloc = np.zeros((NCORES, TPC, NI), np.int16)
    seg = np.full((NCORES, TPC, P, bt), -1.0, np.float32)

    ci = dcore_s
    ti = dtile_s
    srcl1[ci, ti, rank] = l2std[src_s].astype(np.int16)
    srcl2[ci, ti, rank] = l2ag[src_s].astype(np.int16)
    dloc[ci, ti, rank] = dloc_s.astype(np.int16)
    seg[ci, ti, rank % P, rank // P] = dseg_s

    def wrap16(a):                      # [.., NI] -> [.., 128, NI//16]
        w = a.reshape(*a.shape[:-1], NI // 16, 16)
        w = np.swapaxes(w, -1, -2)      # [.., 16, NI//16]
        return np.tile(w, (1, 1, 8, 1)).reshape(*a.shape[:-1], P, NI // 16)

    srcl1_w = wrap16(srcl1)
    srcl2_w = wrap16(srcl2)

    gid = np.full((NCORES, TPC, P), -1.0, np.float32)
    for c in range(NCORES):
        for t in range(TPC):
            b = core_bins[c][t]
            for i, n in enumerate(bin_nodes[b]):
                gid[c, t, i] = batch[n]

    cnt = np.zeros(B, np.float32)
    np.add.at(cnt, batch, 1.0)
    recip_cnt16 = (1.0 / np.maximum(cnt, 1.0)).astype(np.float32).reshape(16, 1)

    return dict(bt=bt, srcl1=srcl1_w, srcl2=srcl2_w, seg=seg,
                gid=gid, recip_cnt16=recip_cnt16, slot_of=slot_of)


def _build_program(bt):
    import concourse.bacc as bacc
    import concourse.bass as bass
    import concourse.mybir as mybir
    import concourse.tile as tile
    from concourse.masks import make_identity
    from concourse import library_config

    f32 = mybir.dt.float32
    bf16 = mybir.dt.bfloat16
    i16 = mybir.dt.int16
    AF = mybir.ActivationFunctionType
    OP = mybir.AluOpType
    IOA = bass.IndirectOffsetOnAxis

    NI = bt * P
    NIW = NI // 16

    nc = bacc.Bacc("TRN2", target_bir_lowering=False, debug=False,
                   enable_asserts=False, num_devices=NCORES)

    # ---------------- inputs ----------------
    t_xs = nc.dram_tensor("xs_host", [NSLOT, XW], bf16, kind="ExternalInput")
    t_xT = nc.dram_tensor("xT_tab", [P, NSLOT], bf16, kind="ExternalInput")
    t_xTloc = nc.dram_tensor("xT_loc", [P, SPC], bf16, kind="ExternalInput")
    t_W1 = nc.dram_tensor("W1b", [P, 1024], bf16, kind="ExternalInput")
    t_V1 = nc.dram_tensor("V1b", [P, 16], bf16, kind="ExternalInput")
    t_W2V2 = nc.dram_tensor("W2V2b", [1024, 528], bf16, kind="ExternalInput")
    t_b1r = nc.dram_tensor("b1_rep", [P, 1024], bf16, kind="ExternalInput")
    t_b2r = nc.dram_tensor("b2_rep", [P, 512], bf16, kind="ExternalInput")
    t_iota = nc.dram_tensor("iota128", [P, P], bf16, kind="ExternalInput")
    t_iota16 = nc.dram_tensor("iota16", [P, 16], bf16, kind="ExternalInput")
    t_rc16 = nc.dram_tensor("recip_cnt16", [16, 1], f32, kind="ExternalInput")
    t_fc1w = nc.dram_tensor("fc1_w", [512, 32], f32, kind="ExternalInput")
    t_fc1b = nc.dram_tensor("fc1_b", [32, 1], f32, kind="ExternalInput")
    t_fc2w = nc.dram_tensor("fc2_w", [32, 10], f32, kind="ExternalInput")
    t_fc2br = nc.dram_tensor("fc2_b_rep", [16, 10], f32, kind="ExternalInput")
    t_srcl1 = nc.dram_tensor("srcl1", [TPC, P, NIW], i16, kind="ExternalInput")
    t_srcl2 = nc.dram_tensor("srcl2", [TPC, P, NIW], i16, kind="ExternalInput")
    t_iota8x = nc.dram_tensor("iota8x", [P, 1024], bf16, kind="ExternalInput")
    t_seg = nc.dram_tensor("seg_m", [TPC, P, bt], f32, kind="ExternalInput")
    t_gid = nc.dram_tensor("gid_m", [TPC, P], f32, kind="ExternalInput")

    t_out = nc.dram_tensor("out", [16, 10], f32, kind="ExternalOutput")

    with tile.TileContext(nc) as tc:
        with (
            tc.tile_pool(name="const", bufs=1) as csb,
            tc.tile_pool(name="dram", bufs=1, space="DRAM") as dr,
        ):
            # DRAM staging
            xs_tab = dr.tile([NSLOT, XW], bf16)
            a1d_loc = dr.tile([SPC, 8], bf16)
            h2p_loc = dr.tile([SPC, HW], bf16)
            h2p_full = dr.tile([NSLOT, HW], bf16)
            AGC = [(0, 4), (4, 7), (7, 10)]
            h2p_chunks = [dr.tile([NCORES * (hi - lo) * P, HW], bf16,
                                  addr_space="Shared", name=f"h2p_chunk{_k}")
                          for _k, (lo, hi) in enumerate(AGC)]
            pool_in = dr.tile([16, 512], f32)
            pool_out = dr.tile([16, 512], f32, addr_space="Shared")

            nc.gpsimd.load_library(library_config.mlp)

            identb = csb.tile([P, P], bf16)
            make_identity(nc, identb[:])
            iota = csb.tile([P, P], bf16)
            nc.sync.dma_start(out=iota[:], in_=t_iota[:])
            iota8x = csb.tile([P, 1024], bf16)
            nc.sync.dma_start(out=iota8x[:], in_=t_iota8x[:])
            a2d_keep = csb.tile([P, 8 * TPC], bf16)
            iota16 = csb.tile([P, 16], bf16)
            nc.sync.dma_start(out=iota16[:], in_=t_iota16[:])
            W1sb = csb.tile([P, 1024], bf16)
            nc.sync.dma_start(out=W1sb[:], in_=t_W1[:])
            V1sb = csb.tile([P, 16], bf16)
            nc.sync.dma_start(out=V1sb[:], in_=t_V1[:])
            W2V2sb = []
            for c in range(8):
                w2c = csb.tile([P, 528], bf16, name=f"w2v2c{c}")
                nc.sync.dma_start(out=w2c[:], in_=t_W2V2[c * P:(c + 1) * P, :])
                W2V2sb.append(w2c)
            b1r = csb.tile([P, 1024], bf16)
            nc.sync.dma_start(out=b1r[:], in_=t_b1r[:])
            b2r = csb.tile([P, 512], bf16)
            nc.sync.dma_start(out=b2r[:], in_=t_b2r[:])
            rc16 = csb.tile([16, 1], f32)
            nc.sync.dma_start(out=rc16[:], in_=t_rc16[:])

            # xs staging copy (x cols; alpha cols written by Phase A)
            nc.sync.dma_start(out=xs_tab[:], in_=t_xs[:])

            # ---------------- Phase A: alpha tables ----------------------
            GRP = 4
            with (
                tc.tile_pool(name="pa_sb", bufs=3) as asb,
                tc.tile_pool(name="pa_ps", bufs=2, space="PSUM") as aps,
            ):
                for g0 in range(0, NCORES * TPC, GRP):
                    sl = slice(g0 * P, (g0 + GRP) * P)
                    xt = asb.tile([P, GRP * P], bf16, name="xt")
                    nc.sync.dma_start(out=xt[:], in_=t_xT[:, sl])
                    al_ps = aps.tile([P, GRP * 16], f32, name="al_ps")
                    al = asb.tile([P, GRP * 16], bf16, name="al")
                    for j in range(GRP):
                        nc.tensor.matmul(al_ps[:, j * 16:(j + 1) * 16],
                                         lhsT=xt[:, j * P:(j + 1) * P],
                                         rhs=V1sb[:], start=True, stop=True)
                        if j % 2 == 0:
                            nc.vector.tensor_copy(
                                out=al[:, j * 16:(j + 1) * 16],
                                in_=al_ps[:, j * 16:(j + 1) * 16])
                        else:
                            nc.scalar.activation(
                                al[:, j * 16:(j + 1) * 16],
                                al_ps[:, j * 16:(j + 1) * 16], AF.Copy)
                    nc.sync.dma_start(
                        out=xs_tab[sl, P:P + 8].rearrange(
                            "(g p) k -> p g k", g=GRP),
                        in_=al[:].rearrange("p (g k) -> p g k", g=GRP)[:, :, 0:8])
                for t0 in range(0, TPC, GRP // 2):
                    G2 = GRP // 2
                    sl = slice(t0 * P, (t0 + G2) * P)
                    xt = asb.tile([P, G2 * P], bf16, name="xt2")
                    nc.sync.dma_start(out=xt[:], in_=t_xTloc[:, sl])
                    al_ps = aps.tile([P, G2 * 16], f32, name="al_ps2")
                    al = asb.tile([P, G2 * 16], bf16, name="al2")
                    for j in range(G2):
                        nc.tensor.matmul(al_ps[:, j * 16:(j + 1) * 16],
                                         lhsT=xt[:, j * P:(j + 1) * P],
                                         rhs=V1sb[:], start=True, stop=True)
                        nc.vector.tensor_copy(out=al[:, j * 16:(j + 1) * 16],
                                              in_=al_ps[:, j * 16:(j + 1) * 16])
                    nc.sync.dma_start(
                        out=a1d_loc[sl, :].rearrange(
                            "(g p) k -> p g k", g=G2),
                        in_=al[:].rearrange("p (g k) -> p g k", g=G2)[:, :, 8:16])

            # ---------------- L1: aggregation + finalize ------------------
            with (
                tc.tile_pool(name="l1_idx", bufs=3) as isb,
                tc.tile_pool(name="l1_g", bufs=2) as gsb,
                tc.tile_pool(name="l1_sb", bufs=3) as lsb,
                tc.tile_pool(name="l1_fin", bufs=2) as fsb,
                tc.tile_pool(name="l1_ps", bufs=1, space="PSUM") as lps,
            ):
                for t in range(TPC):
                    i1 = isb.tile([P, NIW], i16, name="i1")
                    nc.scalar.dma_start(out=i1[:], in_=t_srcl1[t])
                    segm = isb.tile([P, bt], f32, name="segm")
                    nc.scalar.dma_start(out=segm[:], in_=t_seg[t])
                    a1dt = isb.tile([P, 8], bf16, name="a1dt")
                    nc.scalar.dma_start(out=a1dt[:],
                                        in_=a1d_loc[t * P:(t + 1) * P, :])

                    xe = gsb.tile([P, bt, XW], bf16, name="xe")
                    nc.gpsimd.dma_gather(
                        out_ap=xe[:], in_ap=xs_tab[:], idxs_ap=i1[:],
                        num_idxs=NI, num_idxs_reg=NI, elem_size=XW,
                        single_packet=False)

                    agg_ps = lps.tile([P, 1536], f32, name="agg_ps", bufs=1)
                    at_ps = agg_ps[:, 0:1024]
                    den_ps = agg_ps[:, 1024:1032]
                    adp = agg_ps[:, 1040:1040 + bt * 8]

                    # pass 1: one-hots + dst-logit expansion via PE
                    ohs = []
                    for b in range(bt):
                        oh = lsb.tile([P, P], bf16, name=f"oh{b}", tag=f"oh{b}",
                                      bufs=2)
                        nc.vector.tensor_scalar(
                            out=oh[:], in0=iota[:], scalar1=segm[:, b:b + 1],
                            scalar2=None, op0=OP.is_equal)
                        ohs.append(oh)
                        tps = lps.tile([P, P], bf16, name="tps1", tag="tps",
                                       bufs=1)
                        nc.tensor.transpose(out=tps[:], in_=oh[:],
                                            identity=identb[:])
                        ohT = lsb.tile([P, P], bf16, name="ohT", tag="ohT",
                                       bufs=3)
                        if b % 2 == 0:
                            nc.vector.tensor_copy(out=ohT[:], in_=tps[:])
                        else:
                            nc.scalar.activation(ohT[:], tps[:], AF.Copy)
                        nc.tensor.matmul(adp[:, b * 8:(b + 1) * 8], lhsT=ohT[:],
                                         rhs=a1dt[:], start=True, stop=True)

                    esum = lsb.tile([P, bt * 8], bf16, name="esum")
                    nc.vector.tensor_tensor(
                        out=esum[:].rearrange("p (b k) -> p b k", b=bt),
                        in0=xe[:, :, P:P + 8],
                        in1=adp.rearrange("p (b k) -> p b k", b=bt), op=OP.add)
                    lrt = lsb.tile([P, bt * 8], bf16, name="lrt")
                    nc.vector.scalar_tensor_tensor(
                        out=lrt[:], in0=esum[:], scalar=NEG, in1=esum[:],
                        op0=OP.mult, op1=OP.max)
                    exb = lsb.tile([P, bt * 8], bf16, name="exb")
                    nc.scalar.activation(exb[:], lrt[:], AF.Exp)

                    # pass 2: scaled one-hots (one wide stt) + aggregation
                    for b in range(bt):
                        ohx = lsb.tile([P, 1024], bf16, name="ohx", tag="ohx",
                                       bufs=3)
                        nc.vector.scalar_tensor_tensor(
                            out=ohx[:].rearrange("p (g s) -> p g s", g=8),
                            in0=iota8x[:].rearrange("p (g s) -> p g s", g=8),
                            scalar=segm[:, b:b + 1],
                            in1=exb[:, b * 8:(b + 1) * 8].unsqueeze(2)
                                .broadcast_to([P, 8, P]),
                            op0=OP.is_equal, op1=OP.mult)
                        st = (b == 0)
                        sp = (b == bt - 1)
                        xb = xe[:, b, 0:P]
                        nc.tensor.matmul(at_ps[:, 0:512], lhsT=xb,
                                         rhs=ohx[:, 0:512], start=st, stop=sp)
                        nc.tensor.matmul(at_ps[:, 512:1024], lhsT=xb,
                                         rhs=ohx[:, 512:1024], start=st, stop=sp)
                        nc.tensor.matmul(den_ps, lhsT=ohs[b][:],
                                         rhs=exb[:, b * 8:(b + 1) * 8],
                                         start=st, stop=sp)

                    # ---- finalize tile t
                    den = lsb.tile([P, 8], f32, name="den")
                    nc.vector.tensor_scalar_max(out=den[:], in0=den_ps,
                                                scalar1=1e-30)
                    rec = lsb.tile([P, 8], f32, name="rec")
                    nc.vector.reciprocal(out=rec[:], in_=den[:])

                    y = fsb.tile([P, 1024], bf16, name="y")
                    o1t = None
                    for h in range(8):
                        hs = slice(h * P, (h + 1) * P)
                        at_sb = fsb.tile([P, P], bf16, name="at_sb", tag="ats",
                                         bufs=3)
                        if h % 2 == 0:
                            nc.vector.tensor_copy(out=at_sb[:], in_=at_ps[:, hs])
                        else:
                            nc.scalar.activation(at_sb[:], at_ps[:, hs],
                                                 AF.Copy)
                        if h % 4 == 0:
                            o1t = lps.tile([P, 512], f32, name="o1t",
                                           tag="o1p", bufs=2)
                        o1s = o1t[:, (h % 4) * P:(h % 4 + 1) * P]
                        nc.tensor.matmul(o1s, lhsT=at_sb[:], rhs=W1sb[:, hs],
                                         start=True, stop=True)
                        nc.scalar.activation(y[:, hs], o1s, AF.Copy,
                                             scale=rec[:, h:h + 1])
                    y2 = fsb.tile([P, 1024], bf16, name="y2")
                    nc.vector.tensor_add(out=y2[:], in0=y[:], in1=b1r[:])
                    ee = fsb.tile([P, 1024], bf16, name="ee")
                    nc.scalar.activation(ee[:], y2[:], AF.Exp)
                    u = fsb.tile([P, 1024], bf16, name="u")
                    nc.vector.tensor_scalar(out=u[:], in0=ee[:], scalar1=1.0,
                                            scalar2=-1.0, op0=OP.min, op1=OP.add)
                    e1 = fsb.tile([P, 1024], bf16, name="e1")
                    nc.vector.scalar_tensor_tensor(
                        out=e1[:], in0=y2[:], scalar=0.0, in1=u[:],
                        op0=OP.max, op1=OP.add)

                    h2a2_ps = lps.tile([P, 528], f32, name="h2a2_ps", bufs=1)
                    h2_ps = h2a2_ps[:, 0:512]
                    a2_ps = h2a2_ps[:, 512:528]
                    for h in range(8):
                        hs = slice(h * P, (h + 1) * P)
                        tps = lps.tile([P, P], bf16, name="tps", tag="tps",
                                       bufs=1)
                        nc.tensor.transpose(out=tps[:], in_=e1[:, hs],
                                            identity=identb[:])
                        e1T = fsb.tile([P, P], bf16, name="e1T", tag="e1T",
                                       bufs=3)
                        if h % 2 == 0:
                            nc.vector.tensor_copy(out=e1T[:], in_=tps[:])
                        else:
                            nc.scalar.activation(e1T[:], tps[:], AF.Copy)
                        nc.tensor.matmul(h2_ps, lhsT=e1T[:],
                                         rhs=W2V2sb[h][:, 0:512],
                                         start=(h == 0), stop=(h == 7))
                        nc.tensor.matmul(a2_ps, lhsT=e1T[:],
                                         rhs=W2V2sb[h][:, 512:528],
                                         start=(h == 0), stop=(h == 7))
                    hrow = fsb.tile([P, 520], bf16, name="hrow")
                    nc.vector.tensor_copy(out=hrow[:, 0:512], in_=h2_ps)
                    nc.vector.tensor_copy(out=hrow[:, 512:520],
                                          in_=h2a2_ps[:, 512:520])
                    nc.vector.tensor_copy(out=a2d_keep[:, t * 8:(t + 1) * 8],
                                          in_=h2a2_ps[:, 520:528])
                    sl = slice(t * P, (t + 1) * P)
                    nc.sync.dma_start(out=h2p_loc[sl, 0:520], in_=hrow[:])

                    for _k, (lo, hi) in enumerate(AGC):
                        if t != hi - 1:
                            continue
                        nw = hi - lo
                        nc.gpsimd.collective_compute(
                            "AllGather", mybir.AluOpType.bypass,
                            replica_groups=[list(range(NCORES))],
                            ins=[h2p_loc[lo * P:hi * P, :].opt()],
                            outs=[h2p_chunks[_k][:].opt()])
                        full_v = h2p_full[:].rearrange(
                            "(c tt p) w -> c tt p w", c=NCORES,
                            tt=TPC)[:, lo:hi]
                        chunk_v = h2p_chunks[_k][:].rearrange(
                            "(c s p) w -> c s p w", c=NCORES, s=nw)
                        nc.scalar.dma_start(out=full_v, in_=chunk_v)

            # ---------------- L2: aggregation + pool ----------------------
            with (
                tc.tile_pool(name="l2_idx", bufs=3) as isb,
                tc.tile_pool(name="l2_g", bufs=2) as gsb,
                tc.tile_pool(name="l2_sb", bufs=3) as lsb,
                tc.tile_pool(name="l2_fin", bufs=2) as fsb,
                tc.tile_pool(name="l2_ps", bufs=1, space="PSUM") as lps,
            ):
                pool_ps = lps.tile([16, 512], f32, name="pool_ps", bufs=1)
                for t in range(TPC):
                    i1 = isb.tile([P, NIW], i16, name="i1b")
                    nc.scalar.dma_start(out=i1[:], in_=t_srcl2[t])
                    segm = isb.tile([P, bt], f32, name="segm2")
                    nc.scalar.dma_start(out=segm[:], in_=t_seg[t])

                    hg = gsb.tile([P, bt, HW], bf16, name="hg")
                    nc.gpsimd.dma_gather(
                        out_ap=hg[:], in_ap=h2p_full[:], idxs_ap=i1[:],
                        num_idxs=NI, num_idxs_reg=NI, elem_size=HW,
                        single_packet=False)

                    haden_ps = lps.tile([P, 768], f32, name="haden_ps", bufs=2)
                    ha_ps = haden_ps[:, 0:512]
                    den_ps = haden_ps[:, 512:520]
                    adp = haden_ps[:, 528:528 + bt * 8]

                    ohs = []
                    for b in range(bt):
                        oh = lsb.tile([P, P], bf16, name=f"oh2_{b}",
                                      tag=f"oh2_{b}", bufs=2)
                        nc.vector.tensor_scalar(
                            out=oh[:], in0=iota[:], scalar1=segm[:, b:b + 1],
                            scalar2=None, op0=OP.is_equal)
                        ohs.append(oh)
                        tps = lps.tile([P, P], bf16, name="tps2", tag="tps2",
                                       bufs=1)
                        nc.tensor.transpose(out=tps[:], in_=oh[:],
                                            identity=identb[:])
                        ohT = lsb.tile([P, P], bf16, name="ohT2", tag="ohT2",
                                       bufs=3)
                        if b % 2 == 0:
                            nc.vector.tensor_copy(out=ohT[:], in_=tps[:])
                        else:
                            nc.scalar.activation(ohT[:], tps[:], AF.Copy)
                        nc.tensor.matmul(adp[:, b * 8:(b + 1) * 8], lhsT=ohT[:],
                                         rhs=a2d_keep[:, t * 8:(t + 1) * 8],
                                         start=True, stop=True)

                    esum = lsb.tile([P, bt * 8], bf16, name="esum2")
                    nc.vector.tensor_tensor(
                        out=esum[:].rearrange("p (b k) -> p b k", b=bt),
                        in0=hg[:, :, 512:520],
                        in1=adp.rearrange("p (b k) -> p b k", b=bt), op=OP.add)
                    lrt = lsb.tile([P, bt * 8], bf16, name="lrt2")
                    nc.vector.scalar_tensor_tensor(
                        out=lrt[:], in0=esum[:], scalar=NEG, in1=esum[:],
                        op0=OP.mult, op1=OP.max)
                    exb = lsb.tile([P, bt * 8], bf16, name="exb2")
                    nc.scalar.activation(exb[:], lrt[:], AF.Exp)

                    for b in range(bt):
                        msg = lsb.tile([P, 512], bf16, name="msg", tag="msg",
                                       bufs=3)
                        nc.vector.tensor_tensor(
                            out=msg[:].rearrange("p (h c) -> p h c", h=8),
                            in0=hg[:, b, 0:512].rearrange("p (h c) -> p h c", h=8),
                            in1=exb[:, b * 8:(b + 1) * 8].unsqueeze(2)
                                .broadcast_to([P, 8, 64]),
                            op=OP.mult)
                        st = (b == 0)
                        sp = (b == bt - 1)
                        nc.tensor.matmul(ha_ps, lhsT=ohs[b][:], rhs=msg[:],
                                         start=st, stop=sp)
                        nc.tensor.matmul(den_ps, lhsT=ohs[b][:],
                                         rhs=exb[:, b * 8:(b + 1) * 8],
                                         start=st, stop=sp)

                    # ---- finalize tile t
                    den = lsb.tile([P, 8], f32, name="den2")
                    nc.vector.tensor_scalar_max(out=den[:], in0=den_ps,
                                                scalar1=1e-30)
                    rec = lsb.tile([P, 8], f32, name="rec2")
                    nc.vector.reciprocal(out=rec[:], in_=den[:])
                    ey = fsb.tile([P, 512], bf16, name="ey")
                    for h in range(8):
                        hs = slice(h * 64, (h + 1) * 64)
                        nc.scalar.activation(ey[:, hs], ha_ps[:, hs],
                                             AF.Copy, scale=rec[:, h:h + 1])
                    y2 = fsb.tile([P, 512], bf16, name="y2l2")
                    nc.vector.tensor_add(out=y2[:], in0=ey[:], in1=b2r[:])
                    ee = fsb.tile([P, 512], bf16, name="eel2")
                    nc.scalar.activation(ee[:], y2[:], AF.Exp)
                    u = fsb.tile([P, 512], bf16, name="ul2")
                    nc.vector.tensor_scalar(out=u[:], in0=ee[:], scalar1=1.0,
                                            scalar2=-1.0, op0=OP.min, op1=OP.add)
                    e2 = fsb.tile([P, 512], bf16, name="e2")
                    nc.vector.scalar_tensor_tensor(
                        out=e2[:], in0=y2[:], scalar=0.0, in1=u[:],
                        op0=OP.max, op1=OP.add)

                    gidt = lsb.tile([P, 1], f32, name="gidt")
                    nc.sync.dma_start(out=gidt[:], in_=t_gid[t, :, None])
                    gone = lsb.tile([P, 16], bf16, name="gone")
                    nc.vector.tensor_scalar(out=gone[:], in0=iota16[:],
                                            scalar1=gidt[:, 0:1], scalar2=None,
                                            op0=OP.is_equal)
                    nc.tensor.matmul(pool_ps[:], lhsT=gone[:], rhs=e2[:],
                                     start=(t == 0), stop=(t == TPC - 1))

                pool_sb = lsb.tile([16, 512], f32, name="pool_sb")
                nc.vector.tensor_copy(out=pool_sb[:], in_=pool_ps[:])
                nc.sync.dma_start(out=pool_in[:], in_=pool_sb[:])

            nc.gpsimd.collective_compute(
                "AllReduce", mybir.AluOpType.add,
                replica_groups=[list(range(NCORES))],
                ins=[pool_in[:].opt()], outs=[pool_out[:].opt()])

            # ---------------- MLP (replicated) ----------------------------
            with (
                tc.tile_pool(name="pf_sb", bufs=1) as msb,
                tc.tile_pool(name="pf_ps", bufs=1, space="PSUM") as mps,
            ):
                ident32 = msb.tile([16, 16], mybir.dt.float32, name="id32")
                make_identity(nc, ident32[:])
                psb = msb.tile([16, 512], f32, name="psb")
                nc.sync.dma_start(out=psb[:], in_=pool_out[:])
                gt = msb.tile([16, 512], f32, name="gt")
                nc.vector.tensor_scalar_mul(out=gt[:], in0=psb[:],
                                            scalar1=rc16[:, 0:1])
                fc1c = []
                for c in range(4):
                    fw = msb.tile([P, 32], f32, name=f"fc1c{c}")
                    nc.sync.dma_start(out=fw[:], in_=t_fc1w[c * P:(c + 1) * P, :])
                    fc1c.append(fw)
                fb1 = msb.tile([32, 1], f32, name="fb1")
                nc.sync.dma_start(out=fb1[:], in_=t_fc1b[:])
                fw2 = msb.tile([32, 10], f32, name="fw2")
                nc.sync.dma_start(out=fw2[:], in_=t_fc2w[:])
                fb2 = msb.tile([16, 10], f32, name="fb2")
                nc.sync.dma_start(out=fb2[:], in_=t_fc2br[:])

                fc1_ps = mps.tile([32, 16], f32, name="fc1_ps")
                for c in range(4):
                    gtt_ps = mps.tile([P, 16], f32, name="gtt_ps", tag="gtt")
                    nc.tensor.transpose(out=gtt_ps[:],
                                        in_=gt[:, c * P:(c + 1) * P],
                                        identity=ident32[:])
                    gtt = msb.tile([P, 16], f32, name="gtt_sb", tag="gtts")
                    nc.vector.tensor_copy(out=gtt[:], in_=gtt_ps[:])
                    nc.tensor.matmul(fc1_ps[:], lhsT=fc1c[c][:], rhs=gtt[:],
                                     start=(c == 0), stop=(c == 3))
                y1 = msb.tile([32, 16], f32, name="y1")
                nc.vector.tensor_scalar_add(out=y1[:], in0=fc1_ps[:],
                                            scalar1=fb1[:, 0:1])
                en1 = msb.tile([32, 16], f32, name="en1")
                neg1 = msb.tile([32, 16], f32, name="neg1")
                nc.vector.tensor_scalar_min(out=neg1[:], in0=y1[:], scalar1=0.0)
                nc.scalar.activation(en1[:], neg1[:], AF.Exp)
                pm11 = msb.tile([32, 16], f32, name="pm11")
                nc.vector.tensor_scalar(out=pm11[:], in0=y1[:], scalar1=0.0,
                                        scalar2=-1.0, op0=OP.max, op1=OP.add)
                g2 = msb.tile([32, 16], f32, name="g2")
                nc.vector.tensor_add(out=g2[:], in0=pm11[:], in1=en1[:])

                fc2_ps = mps.tile([16, 10], f32, name="fc2_ps")
                nc.tensor.matmul(fc2_ps[:], lhsT=g2[:], rhs=fw2[:],
                                 start=True, stop=True)
                osb = msb.tile([16, 10], f32, name="osb")
                nc.vector.tensor_add(out=osb[:], in0=fc2_ps[:], in1=fb2[:])
                nc.sync.dma_start(out=t_out[:], in_=osb[:])

    nc.compile()
    return nc


def kernel(x, edge_index, batch, W1, att_src1, att_dst1, b1,
           W2, att_src2, att_dst2, b2, fc1_w, fc1_b, fc2_w, fc2_b,
           _trace=False):
    from concourse.bass_utils import run_bass_kernel_spmd
    if _trace:
        try:
            import profile_util
            profile_util.install()
        except Exception:
            pass

    x = np.asarray(x, np.float32)
    W1 = np.asarray(W1, np.float32)
    W2 = np.asarray(W2, np.float32)
    a_s1 = np.asarray(att_src1, np.float32)
    a_d1 = np.asarray(att_dst1, np.float32)
    a_s2 = np.asarray(att_src2, np.float32)
    a_d2 = np.asarray(att_dst2, np.float32)

    pp = _preprocess(np.asarray(edge_index), np.asarray(batch))
    bt = pp['bt']

    if bt not in _PROGRAM_CACHE:
        _PROGRAM_CACHE[bt] = _build_program(bt)
    nc = _PROGRAM_CACHE[bt]

    V1 = np.zeros((P, 16), np.float32)
    V2 = np.zeros((1024, 16), np.float32)
    for h in range(8):
        V1[:, h] = W1[:, h * P:(h + 1) * P] @ a_s1[h]
        V1[:, 8 + h] = W1[:, h * P:(h + 1) * P] @ a_d1[h]
        V2[:, h] = W2[:, h * 64:(h + 1) * 64] @ a_s2[h]
        V2[:, 8 + h] = W2[:, h * 64:(h + 1) * 64] @ a_d2[h]

    slot_of = pp['slot_of']
    xs_host = np.zeros((NSLOT, XW), bf)
    xs_host[slot_of, 0:P] = x.astype(bf)
    xT = np.zeros((P, NSLOT), bf)
    xT[:, slot_of] = x.T.astype(bf)

    W2V2 = np.concatenate([W2, V2], axis=1).astype(bf)    # [1024, 528]

    common = {
        "xs_host": xs_host,
        "xT_tab": xT,
        "W1b": W1.astype(bf),
        "V1b": V1.astype(bf),
        "W2V2b": W2V2,
        "b1_rep": np.tile(np.asarray(b1, np.float32)[None, :], (P, 1)).astype(bf),
        "b2_rep": np.tile(np.asarray(b2, np.float32)[None, :], (P, 1)).astype(bf),
        "iota128": np.tile(np.arange(P, dtype=np.float32)[None, :], (P, 1)).astype(bf),
        "iota8x": np.tile(np.arange(P, dtype=np.float32)[None, :], (P, 8)).astype(bf),
        "iota16": np.tile(np.arange(16, dtype=np.float32)[None, :], (P, 1)).astype(bf),
        "recip_cnt16": pp['recip_cnt16'],
        "fc1_w": np.asarray(fc1_w, np.float32),
        "fc1_b": np.asarray(fc1_b, np.float32).reshape(32, 1),
        "fc2_w": np.asarray(fc2_w, np.float32),
        "fc2_b_rep": np.tile(np.asarray(fc2_b, np.float32)[None, :], (16, 1)),
    }
    in_maps = []
    for c in range(NCORES):
        m = dict(common)
        m["xT_loc"] = np.ascontiguousarray(xT[:, c * SPC:(c + 1) * SPC])
        m["srcl1"] = pp['srcl1'][c]
        m["srcl2"] = pp['srcl2'][c]
        m["seg_m"] = pp['seg'][c]
        m["gid_m"] = pp['gid'][c]
        in_maps.append(m)

    res = run_bass_kernel_spmd(nc, in_maps, list(range(NCORES)),
                               trace=bool(_trace))
    LAST_PROFILE.clear()
    LAST_PROFILE['exec_time_ns'] = res.exec_time_ns
    LAST_PROFILE['results'] = res
    return np.asarray(res.results[0]["out"], np.float32)
